# revision 9
# baseline (speedup 1.0000x reference)
"""StyleGAN2-style modulated 3x3 conv (B=16, C=128, H=W=128) on 8 TRN2 NeuronCores.

Sharding: data-parallel over batch (2 samples/core). Per core:
 - whole zero-padded sample image lives in SBUF as bf16 [128, 130, 130]
   (input DMA'd as fp32 16-row chunks into staging, cast on scalar/gpsimd)
 - modulated weights are bf16 -> FWL weight loads, fully hidden LDWEIGHTS
 - conv = 9 accumulated bf16 matmuls per 4-row psum bank, k-outer over
   8-row groups (2 banks); demod scale on the PSUM->SBUF copy (vector)
 - out DMAs dispatched from the vector queue so the sync queue only paces
   input chunks
"""

import math
from itertools import product

import numpy as np

import concourse.bacc as bacc
import concourse.bass as bass
import concourse.mybir as mybir
import concourse.tile as tile
from concourse.bass_utils import run_bass_kernel_spmd
from concourse.masks import make_identity

B, C, H, W = 16, 128, 128, 128
KK = 3
EPS = 1e-8
N_CORES = 8
S = B // N_CORES          # samples per core
HP, WP = H + 2, W + 2     # zero-padded image dims
RPT = 4                   # output rows per PSUM bank (512 fp32 = 1 bank)
GRP = 2 * RPT             # output rows per matmul group (2 banks)
NG = H // GRP             # groups per sample
CH = 16                   # input rows per staged chunk
NCH = H // CH             # chunks per sample
OTR = 16                  # output rows per store DMA

FP32 = mybir.dt.float32
BF16 = mybir.dt.bfloat16

TAPS = list(product(range(KK), range(KK)))


def build_bass() -> bass.Bass:
    nc = bacc.Bacc(None)
    x_d = nc.dram_tensor("x", [S, C, H, W], FP32, kind="ExternalInput")
    style_d = nc.dram_tensor("style", [S, C], FP32, kind="ExternalInput")
    w_d = nc.dram_tensor("weight", [C, C, KK, KK], FP32, kind="ExternalInput")
    out_d = nc.dram_tensor("out", [S, C, H, W], FP32, kind="ExternalOutput")

    with tile.TileContext(nc) as tc:
        with (
            tc.tile_pool(name="const", bufs=1) as const_pool,
            tc.tile_pool(name="wpool", bufs=1) as wpool,
            tc.tile_pool(name="wmodp", bufs=2) as wmodp,
            tc.tile_pool(name="stage", bufs=2) as stage,
            tc.tile_pool(name="imgp", bufs=2) as imgp,
            tc.tile_pool(name="opool", bufs=3) as opool,
            tc.tile_pool(name="psum_conv", bufs=6, space="PSUM") as psum_conv,
            tc.tile_pool(name="psum_misc", bufs=2, space="PSUM") as psum_misc,
        ):
            # ---- DMAs dispatched first. Weight+style on sync; x chunks are
            # dispatched from the engine that will cast them (scalar for s0,
            # gpsimd for s1) so the sync queue stays free for output stores.
            Wt = wpool.tile([C, C * KK * KK], FP32)
            nc.sync.dma_start(Wt[:], w_d[:].rearrange("o i kh kw -> o (i kh kw)"))
            srow = wpool.tile([S, 2 * C], FP32)
            nc.sync.dma_start(srow[:, 0:C], style_d[:])

            imgs = []
            sts = {b: [] for b in range(S)}

            def chunk_disp(b, ci):
                eng = nc.scalar if b == 0 else nc.gpsimd
                st = stage.tile(
                    [C, CH, W], FP32, name=f"st{b}_{ci}", tag=f"st{b}",
                )
                eng.dma_start(st[:], x_d[b, :, ci * CH:(ci + 1) * CH, :])
                sts[b].append(st)

            def chunk_cast(b, ci):
                dst = imgs[b][:, 1 + ci * CH:1 + (ci + 1) * CH, 1:W + 1]
                if b == 0:
                    nc.scalar.copy(dst, sts[b][ci][:])
                else:
                    nc.gpsimd.tensor_copy(dst, sts[b][ci][:])

            for b in range(S):
                chunk_disp(b, 0)
                chunk_disp(b, 1)

            # ---- gpsimd: identity + image edge zeros ----
            ident = const_pool.tile([128, 128], FP32)
            make_identity(nc, ident)
            for b in range(S):
                img = imgp.tile([C, HP, WP], BF16, name=f"img{b}", tag="img")
                nc.gpsimd.memset(img[:, 0, :], 0.0)
                nc.gpsimd.memset(img[:, HP - 1, :], 0.0)
                nc.gpsimd.memset(img[:, 1:HP - 1, 0], 0.0)
                nc.gpsimd.memset(img[:, 1:HP - 1, WP - 1], 0.0)
                imgs.append(img)
            eps_tile = wpool.tile([C, 1], FP32)
            nc.gpsimd.memset(eps_tile[:], EPS)

            # ---- weight prep, spread across engines ----
            # scalar: wsq = Wt^2 (feeds q_raw)
            wsq = wpool.tile([C, C * KK * KK], FP32)
            nc.scalar.square(wsq[:], Wt[:])

            # gpsimd: q_raw[o,i] = sum_k Wt[o,(ik)]^2 (copy + 8 strided adds)
            q_raw = wpool.tile([C, C], FP32)
            wsq_koi = wsq[:].rearrange("o (i k) -> o k i", k=KK * KK)
            nc.gpsimd.tensor_copy(q_raw[:], wsq_koi[:, 0, :])
            for k in range(1, KK * KK):
                nc.gpsimd.tensor_add(q_raw[:], q_raw[:], wsq_koi[:, k, :])

            # vector: weight-max chain + style normalize
            wmax = wpool.tile([C, 1], FP32)
            nc.vector.tensor_reduce(
                wmax[:], Wt[:], axis=mybir.AxisListType.X,
                op=mybir.AluOpType.max, apply_absolute_value=True,
            )
            smax = wpool.tile([S, 1], FP32)
            nc.vector.tensor_reduce(
                smax[:], srow[:, 0:C], axis=mybir.AxisListType.X,
                op=mybir.AluOpType.max, apply_absolute_value=True,
            )
            sinv = wpool.tile([S, 1], FP32)
            nc.vector.reciprocal(sinv[:], smax[:])
            winv = wpool.tile([C, 1], FP32)
            nc.vector.reciprocal(winv[:], wmax[:])
            nc.vector.tensor_scalar_mul(winv[:], winv[:], 1.0 / math.sqrt(C * KK * KK))
            tsq = wpool.tile([C, 1], FP32)
            nc.vector.tensor_mul(tsq[:], winv[:], winv[:])

            # gpsimd: s = style*sinv, s2 = s^2 (rows 0..S-1)
            nc.gpsimd.tensor_scalar_mul(srow[:, 0:C], srow[:, 0:C], sinv[:])
            nc.gpsimd.tensor_mul(srow[:, C:2 * C], srow[:, 0:C], srow[:, 0:C])

            # vector: wn = Wt * winv
            wn = wpool.tile([C, C * KK * KK], FP32)
            nc.vector.tensor_scalar_mul(wn[:], Wt[:], winv[:])

            # PE: transpose style rows -> per-partition columns
            scol = wpool.tile([C, S], FP32)
            s2col = wpool.tile([C, S], FP32)
            pt_s = psum_misc.tile([C, S], FP32, name="pt_s", tag="mps")
            nc.tensor.transpose(pt_s[:], srow[:, 0:C], ident[0:S, 0:S])
            nc.vector.tensor_copy(scol[:], pt_s[:])
            pt_s2 = psum_misc.tile([C, S], FP32, name="pt_s2", tag="mps")
            nc.tensor.transpose(pt_s2[:], srow[:, C:2 * C], ident[0:S, 0:S])
            nc.vector.tensor_copy(s2col[:], pt_s2[:])

            # PE: transpose wn -> wn_t[i, k*128+o] (9 transposes, pipelined)
            wn_t = wpool.tile([C, KK * KK * C], FP32)
            wn_koi = wn[:].rearrange("o (i k) -> o k i", k=KK * KK)
            for k in range(KK * KK):
                pt = psum_misc.tile([128, 128], FP32, name=f"pt{k}", tag="mps")
                nc.tensor.transpose(pt[:], wn_koi[:, k, :], ident[:])
                nc.vector.tensor_copy(wn_t[:, k * C:(k + 1) * C], pt[:])

            # PE: q_raw^T, then coe_raw[o,b] = sum_i q_raw[o,i] * s2[i,b]
            q_t = wpool.tile([C, C], FP32)
            pt_q = psum_misc.tile([128, 128], FP32, name="pt_q", tag="mps")
            nc.tensor.transpose(pt_q[:], q_raw[:], ident[:])
            nc.vector.tensor_copy(q_t[:], pt_q[:])
            ps_coe = psum_misc.tile([C, S], FP32, name="ps_coe", tag="mps")
            nc.tensor.matmul(ps_coe[:], q_t[:], s2col[:], start=True, stop=True)

            # vector: per-sample modulated weights in bf16 (before coe chain
            # so the PE can start; coe only gates the first demod)
            wmods = []
            for b in range(S):
                wmod = wmodp.tile([C, KK * KK * C], BF16, name=f"wmod{b}", tag="wmod")
                nc.vector.tensor_scalar_mul(wmod[:], wn_t[:], scol[:, b:b + 1])
                wmods.append(wmod)

            # ---- x chunks: casts into the padded bf16 images, paced by the
            # staging pool; the next dispatch is emitted right after the cast
            # that frees its slot.
            coe_s = wpool.tile([C, S], FP32)
            for ci in range(NCH):
                for b in range(S):
                    chunk_cast(b, ci)
                    if ci + 2 < NCH:
                        chunk_disp(b, ci + 2)
                if ci == 1:
                    # emit the sqrt after two casts so the scalar queue gets
                    # the early casts out first; coe = 1/sqrt(raw*winv^2+eps)
                    nc.scalar.activation(
                        coe_s[:], ps_coe[:], mybir.ActivationFunctionType.Sqrt,
                        bias=eps_tile[:], scale=tsq[:, 0:1],
                    )

            coe = wpool.tile([C, S], FP32)
            nc.vector.reciprocal(coe[:], coe_s[:])

            # ---- conv: per sample, 16 groups of 8 rows (2 psum banks) ----
            for b in range(S):
                img = imgs[b]
                wmod = wmods[b]
                for g in range(NG):
                    y0 = GRP * g
                    ps0 = psum_conv.tile([C, RPT * W], FP32, name="ps0", tag="ps")
                    ps1 = psum_conv.tile([C, RPT * W], FP32, name="ps1", tag="ps")
                    for idx, (dy, dx) in enumerate(TAPS):
                        k = dy * KK + dx
                        lhs = wmod[:, k * C:(k + 1) * C]
                        st_, sp_ = idx == 0, idx == KK * KK - 1
                        nc.tensor.matmul(
                            ps0[:], lhs,
                            img[:, y0 + dy:y0 + dy + RPT, dx:dx + W],
                            start=st_, stop=sp_,
                        )
                        nc.tensor.matmul(
                            ps1[:], lhs,
                            img[:, y0 + RPT + dy:y0 + RPT + dy + RPT, dx:dx + W],
                            start=st_, stop=sp_,
                        )
                    off = (g % 2) * GRP
                    if g % 2 == 0:
                        ot = opool.tile([C, OTR, W], FP32, name="ot", tag="ot")
                    nc.vector.tensor_scalar_mul(
                        ot[:, off:off + RPT, :],
                        ps0[:].rearrange("c (r w) -> c r w", r=RPT),
                        coe[:, b:b + 1],
                    )
                    nc.vector.tensor_scalar_mul(
                        ot[:, off + RPT:off + GRP, :],
                        ps1[:].rearrange("c (r w) -> c r w", r=RPT),
                        coe[:, b:b + 1],
                    )
                    if g % 2 == 1:
                        r0 = OTR * (g // 2)
                        nc.sync.dma_start(
                            out_d[b, :, r0:r0 + OTR, :], ot[:],
                        )

    nc.compile()
    return nc


_CACHED = {}


def kernel(x: np.ndarray, style: np.ndarray, weight: np.ndarray, trace: bool = False):
    x = np.ascontiguousarray(x, dtype=np.float32)
    style = np.ascontiguousarray(style, dtype=np.float32)
    weight = np.ascontiguousarray(weight, dtype=np.float32)

    if "nc" not in _CACHED:
        _CACHED["nc"] = build_bass()
    nc = _CACHED["nc"]

    in_maps = [
        {
            "x": x[i * S:(i + 1) * S],
            "style": style[i * S:(i + 1) * S],
            "weight": weight,
        }
        for i in range(N_CORES)
    ]
    res = run_bass_kernel_spmd(
        nc, in_maps, core_ids=list(range(N_CORES)), trace=trace,
    )
    out = np.concatenate([r["out"] for r in res.results], axis=0)
    if trace:
        kernel.last_results = res
    return out


# revision 11
# speedup vs baseline: 1.1076x; 1.1076x over previous
"""StyleGAN2-style modulated 3x3 conv (B=16, C=128, H=W=128) on 8 TRN2 NeuronCores.

Sharding: data-parallel over batch (2 samples/core). Per core:
 - whole zero-padded sample image lives in SBUF as bf16 [128, 130, 130]
   (input DMA'd as fp32 16-row chunks into staging, cast on scalar/gpsimd)
 - modulated weights are bf16 -> FWL weight loads, fully hidden LDWEIGHTS
 - conv = 9 accumulated bf16 matmuls per 4-row psum bank, k-outer over
   8-row groups (2 banks); demod scale on the PSUM->SBUF copy (vector)
 - out DMAs dispatched from the vector queue so the sync queue only paces
   input chunks
"""

import math
from itertools import product

import numpy as np

import concourse.bacc as bacc
import concourse.bass as bass
import concourse.mybir as mybir
import concourse.tile as tile
from concourse.bass_utils import run_bass_kernel_spmd
from concourse.masks import make_identity

B, C, H, W = 16, 128, 128, 128
KK = 3
EPS = 1e-8
N_CORES = 8
S = B // N_CORES          # samples per core
HP, WP = H + 2, W + 2     # zero-padded image dims
RPT = 4                   # output rows per PSUM bank (512 fp32 = 1 bank)
GRP = 2 * RPT             # output rows per matmul group (2 banks)
NG = H // GRP             # groups per sample
CH = 16                   # input rows per staged chunk
NCH = H // CH             # chunks per sample
OTR = 16                  # output rows per store DMA

FP32 = mybir.dt.float32
BF16 = mybir.dt.bfloat16

TAPS = list(product(range(KK), range(KK)))


def build_bass() -> bass.Bass:
    nc = bacc.Bacc(None)
    x_d = nc.dram_tensor("x", [S, C, H, W], FP32, kind="ExternalInput")
    style_d = nc.dram_tensor("style", [S, C], FP32, kind="ExternalInput")
    w_d = nc.dram_tensor("weight", [C, C, KK, KK], FP32, kind="ExternalInput")
    out_d = nc.dram_tensor("out", [S, C, H, W], FP32, kind="ExternalOutput")

    with tile.TileContext(nc) as tc:
        with (
            tc.tile_pool(name="const", bufs=1) as const_pool,
            tc.tile_pool(name="wpool", bufs=1) as wpool,
            tc.tile_pool(name="wmodp", bufs=2) as wmodp,
            tc.tile_pool(name="stage", bufs=2) as stage,
            tc.tile_pool(name="imgp", bufs=2) as imgp,
            tc.tile_pool(name="opool", bufs=3) as opool,
            tc.tile_pool(name="psum_conv", bufs=6, space="PSUM") as psum_conv,
            tc.tile_pool(name="psum_misc", bufs=2, space="PSUM") as psum_misc,
        ):
            # ---- DMAs dispatched first. Weight+style on sync; x chunks are
            # dispatched from the engine that will cast them (scalar for s0,
            # gpsimd for s1) so the sync queue stays free for output stores.
            srow = wpool.tile([S, 2 * C], FP32)
            nc.sync.dma_start(srow[:, 0:C], style_d[:])
            Wt = wpool.tile([C, C * KK * KK], FP32)
            nc.sync.dma_start(Wt[:], w_d[:].rearrange("o i kh kw -> o (i kh kw)"))

            imgs = []
            sts = {b: [] for b in range(S)}

            def chunk_disp(b, ci):
                eng = nc.scalar if b == 0 else nc.gpsimd
                st = stage.tile(
                    [C, CH, W], FP32, name=f"st{b}_{ci}", tag=f"st{b}",
                )
                eng.dma_start(st[:], x_d[b, :, ci * CH:(ci + 1) * CH, :])
                sts[b].append(st)

            def chunk_cast(b, ci):
                dst = imgs[b][:, 1 + ci * CH:1 + (ci + 1) * CH, 1:W + 1]
                nc.scalar.copy(dst, sts[b][ci][:])

            for b in range(S):
                chunk_disp(b, 0)
                chunk_disp(b, 1)

            # ---- gpsimd: identity + image edge zeros ----
            ident = const_pool.tile([128, 128], FP32)
            make_identity(nc, ident)
            for b in range(S):
                img = imgp.tile([C, HP, WP], BF16, name=f"img{b}", tag="img")
                nc.gpsimd.memset(img[:, 0, :], 0.0)
                nc.gpsimd.memset(img[:, HP - 1, :], 0.0)
                nc.gpsimd.memset(img[:, 1:HP - 1, 0], 0.0)
                nc.gpsimd.memset(img[:, 1:HP - 1, WP - 1], 0.0)
                imgs.append(img)
            eps_tile = wpool.tile([C, 1], FP32)
            nc.gpsimd.memset(eps_tile[:], EPS)

            # ---- weight prep, spread across engines ----
            # scalar: wsq = Wt^2 (feeds q_raw for the demod coefficient)
            wsq = wpool.tile([C, C * KK * KK], FP32)
            nc.scalar.square(wsq[:], Wt[:])

            # vector: style norm first (style lands early), then weight max
            smax = wpool.tile([S, 1], FP32)
            nc.vector.tensor_reduce(
                smax[:], srow[:, 0:C], axis=mybir.AxisListType.X,
                op=mybir.AluOpType.max, apply_absolute_value=True,
            )
            sinv = wpool.tile([S, 1], FP32)
            nc.vector.reciprocal(sinv[:], smax[:])

            # gpsimd: s = style*sinv, s2 = s^2, then q_raw[o,i] = sum_k Wt^2
            nc.gpsimd.tensor_scalar_mul(srow[:, 0:C], srow[:, 0:C], sinv[:])
            nc.gpsimd.tensor_mul(srow[:, C:2 * C], srow[:, 0:C], srow[:, 0:C])
            q_raw = wpool.tile([C, C], FP32)
            wsq_koi = wsq[:].rearrange("o (i k) -> o k i", k=KK * KK)
            nc.gpsimd.tensor_copy(q_raw[:], wsq_koi[:, 0, :])
            for k in range(1, KK * KK):
                nc.gpsimd.tensor_add(q_raw[:], q_raw[:], wsq_koi[:, k, :])

            # PE: transpose RAW Wt -> wn_t[i, k*128+o]; the 1/(sqrt(ikk)*wmax)
            # normalization is a per-o scale folded into the demod coefficient
            wn_t = wpool.tile([C, KK * KK * C], FP32)
            wt_koi = Wt[:].rearrange("o (i k) -> o k i", k=KK * KK)
            for k in range(KK * KK):
                pt = psum_misc.tile([128, 128], FP32, name=f"pt{k}", tag="mps")
                nc.tensor.transpose(pt[:], wt_koi[:, k, :], ident[:])
                nc.vector.tensor_copy(wn_t[:, k * C:(k + 1) * C], pt[:])

            # PE: style rows -> per-partition columns
            scol = wpool.tile([C, S], FP32)
            s2col = wpool.tile([C, S], FP32)
            pt_s = psum_misc.tile([C, S], FP32, name="pt_s", tag="mps")
            nc.tensor.transpose(pt_s[:], srow[:, 0:C], ident[0:S, 0:S])
            nc.vector.tensor_copy(scol[:], pt_s[:])
            pt_s2 = psum_misc.tile([C, S], FP32, name="pt_s2", tag="mps")
            nc.tensor.transpose(pt_s2[:], srow[:, C:2 * C], ident[0:S, 0:S])

            # vector: per-sample modulated weights in bf16, then the wmax
            # chain (only needed for the demod scale, so it comes after)
            wmods = []
            for b in range(S):
                wmod = wmodp.tile([C, KK * KK * C], BF16, name=f"wmod{b}", tag="wmod")
                nc.vector.tensor_scalar_mul(wmod[:], wn_t[:], scol[:, b:b + 1])
                wmods.append(wmod)
            nc.vector.tensor_copy(s2col[:], pt_s2[:])
            wmax = wpool.tile([C, 1], FP32)
            nc.vector.tensor_reduce(
                wmax[:], Wt[:], axis=mybir.AxisListType.X,
                op=mybir.AluOpType.max, apply_absolute_value=True,
            )
            winv = wpool.tile([C, 1], FP32)
            nc.vector.reciprocal(winv[:], wmax[:])
            nc.vector.tensor_scalar_mul(winv[:], winv[:], 1.0 / math.sqrt(C * KK * KK))
            tsq = wpool.tile([C, 1], FP32)
            nc.vector.tensor_mul(tsq[:], winv[:], winv[:])

            # ---- helpers for conv groups: matmuls and (deferred) demods
            ot_holder = [None]

            def emit_group_mms(b, g):
                img, wmod = imgs[b], wmods[b]
                y0 = GRP * g
                ps0 = psum_conv.tile([C, RPT * W], FP32, name="ps0", tag="ps")
                ps1 = psum_conv.tile([C, RPT * W], FP32, name="ps1", tag="ps")
                for idx, (dy, dx) in enumerate(TAPS):
                    k = dy * KK + dx
                    lhs = wmod[:, k * C:(k + 1) * C]
                    st_, sp_ = idx == 0, idx == KK * KK - 1
                    nc.tensor.matmul(
                        ps0[:], lhs,
                        img[:, y0 + dy:y0 + dy + RPT, dx:dx + W],
                        start=st_, stop=sp_,
                    )
                    nc.tensor.matmul(
                        ps1[:], lhs,
                        img[:, y0 + RPT + dy:y0 + RPT + dy + RPT, dx:dx + W],
                        start=st_, stop=sp_,
                    )
                return ps0, ps1

            def emit_group_demod(b, g, ps0, ps1):
                off = (g % 2) * GRP
                if g % 2 == 0:
                    ot_holder[0] = opool.tile([C, OTR, W], FP32, name="ot", tag="ot")
                ot = ot_holder[0]
                nc.vector.tensor_scalar_mul(
                    ot[:, off:off + RPT, :],
                    ps0[:].rearrange("c (r w) -> c r w", r=RPT),
                    dscale[:, b:b + 1],
                )
                nc.vector.tensor_scalar_mul(
                    ot[:, off + RPT:off + GRP, :],
                    ps1[:].rearrange("c (r w) -> c r w", r=RPT),
                    dscale[:, b:b + 1],
                )
                if g % 2 == 1:
                    r0 = OTR * (g // 2)
                    nc.sync.dma_start(out_d[b, :, r0:r0 + OTR, :], ot[:])

            dscale = wpool.tile([C, S], FP32)

            # ---- early casts (2 chunks per sample) so the PE can start ----
            for ci in range(2):
                for b in range(S):
                    chunk_cast(b, ci)
                    chunk_disp(b, ci + 2)

            # first two groups of sample 0: matmuls only, demods deferred
            # until the demod scale exists
            pend = [(0, g) + emit_group_mms(0, g) for g in (0, 1)]

            # PE: q_raw^T then coe_raw[o,b] = sum_i q_raw[o,i]*s2[i,b]
            # (emitted after the first conv groups so the wait on q_raw
            # never blocks conv matmuls in the tensor queue)
            q_t = wpool.tile([C, C], FP32)
            pt_q = psum_misc.tile([128, 128], FP32, name="pt_q", tag="mps")
            nc.tensor.transpose(pt_q[:], q_raw[:], ident[:])
            nc.vector.tensor_copy(q_t[:], pt_q[:])
            ps_coe = psum_misc.tile([C, S], FP32, name="ps_coe", tag="mps")
            nc.tensor.matmul(ps_coe[:], q_t[:], s2col[:], start=True, stop=True)

            # scalar: coe_s = sqrt(coe_raw*winv^2 + eps)
            coe_s = wpool.tile([C, S], FP32)
            nc.scalar.activation(
                coe_s[:], ps_coe[:], mybir.ActivationFunctionType.Sqrt,
                bias=eps_tile[:], scale=tsq[:, 0:1],
            )

            # ---- remaining casts, paced by the staging pool ----
            for ci in range(2, NCH):
                for b in range(S):
                    chunk_cast(b, ci)
                    if ci + 2 < NCH:
                        chunk_disp(b, ci + 2)

            # demod scale = winv / sqrt(...)  (per output channel o, sample b)
            coe = wpool.tile([C, S], FP32)
            nc.vector.reciprocal(coe[:], coe_s[:])
            nc.vector.tensor_scalar_mul(dscale[:], coe[:], winv[:])

            # flush deferred demods, then the rest of the conv
            for b, g, ps0, ps1 in pend:
                emit_group_demod(b, g, ps0, ps1)
            for g in range(2, NG):
                ps0, ps1 = emit_group_mms(0, g)
                emit_group_demod(0, g, ps0, ps1)
            for g in range(NG):
                ps0, ps1 = emit_group_mms(1, g)
                emit_group_demod(1, g, ps0, ps1)

    nc.compile()
    return nc


_CACHED = {}


def kernel(x: np.ndarray, style: np.ndarray, weight: np.ndarray, trace: bool = False):
    x = np.ascontiguousarray(x, dtype=np.float32)
    style = np.ascontiguousarray(style, dtype=np.float32)
    weight = np.ascontiguousarray(weight, dtype=np.float32)

    if "nc" not in _CACHED:
        _CACHED["nc"] = build_bass()
    nc = _CACHED["nc"]

    in_maps = [
        {
            "x": x[i * S:(i + 1) * S],
            "style": style[i * S:(i + 1) * S],
            "weight": weight,
        }
        for i in range(N_CORES)
    ]
    res = run_bass_kernel_spmd(
        nc, in_maps, core_ids=list(range(N_CORES)), trace=trace,
    )
    out = np.concatenate([r["out"] for r in res.results], axis=0)
    if trace:
        kernel.last_results = res
    return out
